# revision 13
# baseline (speedup 1.0000x reference)
"""InstanceConsistencyLoss Trainium2 kernel.

Strategy (data-parallel over batch): 8 images -> 8 NeuronCores, one image per
core.  On the host, features are relaid out per image to (P=H*W, 130) bf16
where columns 0..127 are the channels, column 128 is a slot the kernel fills
with g[p] = sum_c f[p,c]^2, and column 129 is constant 1.  On device, for each
128-pixel chunk the vector engine builds a (128, 256) bf16 one-hot of the
instance id against iota 1..256 (background id 0 matches nothing and is
dropped, exactly as the reference drops segment 0), and the tensor engine
accumulates onehot.T @ [f | g | 1] into two persistent PSUM tiles — giving
per-segment [sum_f, sum_f2_total, count] for segments 1..128 and 129..256.
A short epilogue computes V_s = (G_s - Q_s/cnt_s)/cnt_s, masks empty segments,
and reduces to per-image [sum_V, n_instances] via a ones-matmul.  The host
finishes with L = mean_b(sum_V_b / n_b), 16 scalars of work.
"""

import os
import sys

import numpy as np

sys.path.insert(0, "/opt/trn_rl_repo")

import ml_dtypes  # noqa: E402

BF = ml_dtypes.bfloat16

B, C, H, W = 8, 128, 512, 512
P = H * W              # 262144 pixels per image
CHUNK = 128            # pixels per matmul contraction
KB = 32                # chunks per DMA block
BLK = CHUNK * KB       # 1024 pixels per block
NBLK = P // BLK        # blocks
NCHUNK = P // CHUNK    # 2048 chunks
RC = C + 2             # DRAM columns: 128 features + ones + zero pad
FOLD = 32              # f^2 folded to this many columns (PE sums them)
RS = C + 2 + FOLD      # SBUF rhs columns: f | ones | pad | f2fold
NSEG = 256             # foreground ids 1..256

_STATE = {}


def _build_program():
    import concourse.bass as bass
    import concourse.bacc as bacc
    import concourse.mybir as mybir
    from concourse.tile import TileContext

    fp32 = mybir.dt.float32
    bf16 = mybir.dt.bfloat16
    AX = mybir.AxisListType
    ALU = mybir.AluOpType
    ACTF = mybir.ActivationFunctionType

    nc = bacc.Bacc("TRN2", target_bir_lowering=False, debug=False)

    f_dram = nc.dram_tensor("f", (P, RS), bf16, kind="ExternalInput").ap()
    ids_dram = nc.dram_tensor("ids", (128, NCHUNK), fp32, kind="ExternalInput").ap()
    iota_dram = nc.dram_tensor("iota", (128, NSEG), bf16, kind="ExternalInput").ap()
    ones_dram = nc.dram_tensor("ones", (128, 1), fp32, kind="ExternalInput").ap()
    out_dram = nc.dram_tensor("out", (2, 1), fp32, kind="ExternalOutput").ap()

    with TileContext(nc) as tc:
        with (
            tc.tile_pool(name="const", bufs=1) as cpool,
            tc.tile_pool(name="fio", bufs=4) as fpool,
            tc.tile_pool(name="sq", bufs=3) as sqpool,
            tc.tile_pool(name="oh", bufs=6) as ohpool,
            tc.tile_pool(name="ep", bufs=2) as eppool,
            tc.tile_pool(name="acc", bufs=1, space="PSUM") as ppool,
            tc.tile_pool(name="fin", bufs=1, space="PSUM") as pfpool,
        ):
            ids_t = cpool.tile([128, NCHUNK], fp32)
            nc.sync.dma_start(ids_t[:], ids_dram)
            iota_t = cpool.tile([128, NSEG], bf16)
            nc.sync.dma_start(iota_t[:], iota_dram)
            ones_t = cpool.tile([128, 1], fp32)
            nc.sync.dma_start(ones_t[:], ones_dram)

            acc_lo = ppool.tile([128, RS], fp32)
            acc_hi = ppool.tile([128, RS], fp32)

            for q in range(NBLK):
                fblk = fpool.tile([128, KB, RS], bf16, tag="fblk")
                src = f_dram[q * BLK:(q + 1) * BLK, :].rearrange(
                    "(p k) c -> p k c", k=KB)
                nc.sync.dma_start(fblk[:], src)

                f2 = sqpool.tile([128, KB, C], bf16, tag="f2")
                nc.scalar.activation(f2[:], fblk[:, :, 0:C], ACTF.Square)
                # fold 128->64 on the otherwise-idle GPSIMD, 64->32 on DVE;
                # PE sums the remaining 32 inside the segment matmul
                f2h = sqpool.tile([128, KB, 64], bf16, tag="f2h")
                with nc.allow_low_precision(reason="f2 partials stay bf16"):
                    nc.gpsimd.tensor_add(f2h[:], f2[:, :, 0:64],
                                         f2[:, :, 64:C])
                    nc.vector.tensor_add(fblk[:, :, RC:RS],
                                         f2h[:, :, 0:32], f2h[:, :, 32:64])

                for k in range(KB):
                    j = q * KB + k
                    oh = ohpool.tile([128, NSEG], bf16, tag="oh")
                    nc.vector.tensor_scalar(
                        oh[:], iota_t[:], ids_t[:, j:j + 1], None, ALU.is_equal)
                    first = j == 0
                    last = j == NCHUNK - 1
                    nc.tensor.matmul(acc_lo[:], oh[:, 0:128], fblk[:, k, :],
                                     start=first, stop=last)
                    nc.tensor.matmul(acc_hi[:], oh[:, 128:256], fblk[:, k, :],
                                     start=first, stop=last)

            fin = pfpool.tile([2, 1], fp32)
            for half, acc in ((0, acc_lo), (1, acc_hi)):
                sqs = eppool.tile([128, C], fp32, tag="sqs")
                qsum = eppool.tile([128, 1], fp32, tag="qsum")
                nc.scalar.activation(sqs[:], acc[:, 0:C], ACTF.Square,
                                     accum_out=qsum[:])
                gsum = eppool.tile([128, 1], fp32, tag="gsum")
                nc.vector.tensor_reduce(gsum[:], acc[:, RC:RS], axis=AX.X,
                                        op=ALU.add)
                cnt_s = eppool.tile([128, 1], fp32, tag="cnt_s")
                nc.vector.tensor_scalar_max(cnt_s[:], acc[:, C:C + 1], 1.0)
                rec = eppool.tile([128, 1], fp32, tag="rec")
                nc.vector.reciprocal(rec[:], cnt_s[:])
                vres = eppool.tile([128, 2], fp32, tag="vres")
                nc.vector.tensor_scalar(
                    vres[:, 1:2], acc[:, C:C + 1], 0.5, None, ALU.is_gt)
                t1 = eppool.tile([128, 1], fp32, tag="t1")
                nc.vector.tensor_mul(t1[:], qsum[:], rec[:])
                t2 = eppool.tile([128, 1], fp32, tag="t2")
                nc.vector.tensor_sub(t2[:], gsum[:], t1[:])
                t3 = eppool.tile([128, 1], fp32, tag="t3")
                nc.vector.tensor_mul(t3[:], t2[:], rec[:])
                nc.vector.tensor_mul(vres[:, 0:1], t3[:], vres[:, 1:2])
                nc.tensor.matmul(fin[:], vres[:], ones_t[:],
                                 start=(half == 0), stop=(half == 1))

            fin_sb = eppool.tile([2, 1], fp32, tag="fin_sb")
            nc.scalar.copy(fin_sb[:], fin[:])
            nc.sync.dma_start(out_dram, fin_sb[:])

    nc.compile()
    return nc


def _get_program():
    if "nc" not in _STATE:
        _STATE["nc"] = _build_program()
    return _STATE["nc"]


def _prep_inputs(features, instance_ids):
    """Host-side relayout/sharding: one in_map per core (= per image)."""
    features = np.asarray(features)
    instance_ids = np.asarray(instance_ids)

    # (B, C, H, W) -> (B, P, C) bf16, padded to (B, P, RC) with g-slot + ones
    f_pc = np.ascontiguousarray(
        features.reshape(B, C, P).transpose(0, 2, 1)).astype(BF)
    f_pad = np.zeros((B, P, RS), dtype=BF)
    f_pad[:, :, :C] = f_pc
    f_pad[:, :, C] = BF(1.0)      # ones column -> per-segment count
    # cols C+1..RS-1 stay zero: pad + fold slots (overwritten on device);
    # full-width rows keep the DMA contiguous per partition

    # chunk j = q*KB + k holds pixels q*BLK + p*KB + k (p = partition)
    ids_prep = instance_ids.reshape(B, NBLK, 128, KB).transpose(0, 2, 1, 3)
    ids_prep = np.ascontiguousarray(ids_prep.reshape(B, 128, NCHUNK)).astype(
        np.float32)

    iota = np.tile(np.arange(1, NSEG + 1, dtype=np.float32)[None, :],
                   (128, 1)).astype(BF)
    ones = np.ones((128, 1), dtype=np.float32)

    in_maps = []
    for b in range(B):
        in_maps.append({
            "f": f_pad[b],
            "ids": ids_prep[b],
            "iota": iota,
            "ones": ones,
        })
    return in_maps


def _postprocess(results):
    total = 0.0
    for res in results:
        out = np.asarray(res["out"], dtype=np.float64).reshape(2)
        sum_v, n_inst = out[0], out[1]
        if n_inst > 0:
            total += sum_v / n_inst
    return np.float32(total / B)


def kernel(features, instance_ids, _trace=False, _trace_kwargs=None):
    from concourse import bass_utils

    nc = _get_program()
    in_maps = _prep_inputs(features, instance_ids)
    kw = dict(_trace_kwargs or {})
    res = bass_utils.run_bass_kernel_spmd(
        nc, in_maps, core_ids=list(range(B)), trace=_trace, **kw)
    out = _postprocess(res.results)
    if _trace:
        return out, res
    return out


if __name__ == "__main__":
    rng = np.random.default_rng(0)
    feats = rng.standard_normal((B, C, H, W), dtype=np.float32)
    ids = rng.integers(0, 257, size=(B, H, W)).astype(np.int32)
    print(kernel(feats, ids))
